# revision 11
# baseline (speedup 1.0000x reference)
"""Trainium2 Bass kernel for the ExplicitV2C GNN layer (GATv2 message passing).

Strategy (8-core SPMD, no collectives):
  * Host: permute nodes into 512 degree-balanced bins of 128 nodes; group
    edges by destination bin; pad each bin to S subtiles of 128 edges.
    Each core owns 64 bins (8192 dst nodes) and all edges targeting them.
  * Device per core:
      Phase 1 (replicated): LLR fusion (Linear+LN+ReLU+mask) over ALL nodes;
        writes the full bf16 x_w table to core-local DRAM (gather source).
      Phase 2 (edges, sharded): batched indirect-DMA gathers of x_w rows
        (2560 rows per DMA op, including each window's own dst nodes), GATv2
        scores with bf16 matmuls, leaky_relu as 0.2*z + 0.8*relu(z),
        segment softmax + weighted aggregation via one-hot matmuls in PSUM.
      Phase 3 (nodes, sharded): degree gate + final LayerNorm; the degree
        embedding term is fetched with a single dma_gather op.
  * Host: reorder the output shards, undo the node permutation.
"""

import os
import sys

sys.path.insert(0, "/opt/trn_rl_repo")

import numpy as np
import ml_dtypes

import concourse.bass as bass
import concourse.bacc as bacc
import concourse.mybir as mybir
import concourse.tile as tile
from concourse.bass import IndirectOffsetOnAxis
from concourse.bass_utils import run_bass_kernel_spmd

F32 = mybir.dt.float32
BF16 = mybir.dt.bfloat16
I32 = mybir.dt.int32
I16 = mybir.dt.int16
AX = mybir.AxisListType
OP = mybir.AluOpType
AF = mybir.ActivationFunctionType

P = 128
NCORES = 8
LN_EPS = 1e-5
SM_EPS = 1e-16
BF = ml_dtypes.bfloat16


class Cfg:
    def __init__(self, N=65536, E=262144, S_SUB=4):
        self.N, self.E, self.S_SUB = N, E, S_SUB
        self.BINS = N // P                       # node bins total (512)
        self.BPC = self.BINS // NCORES           # windows per core (64)
        self.NSHARD = N // NCORES                # nodes per core (8192)
        self.SLOTS = S_SUB * P                   # edge slots per bin
        self.NSLAB = N // 512                    # phase-1 slabs (128)
        self.CH = 16                             # gather chunks per core
        self.WPC = self.BPC // self.CH           # windows per chunk (4)
        self.CPW = S_SUB + 1                     # gather cols per window


# ----------------------------------------------------------------------------
# Host-side preprocessing
# ----------------------------------------------------------------------------

def _balance_bins(deg_in, N, BINS, target):
    """LPT assignment: nodes by in-degree descending onto the lightest bin
    that still has free slots; every bin gets exactly P nodes."""
    import heapq
    order = np.argsort(-deg_in, kind="stable")
    bin_of = np.empty(N, np.int64)
    slot_of = np.empty(N, np.int64)
    heap = [(0, 0, b) for b in range(BINS)]
    heapq.heapify(heap)
    for n in order:
        while True:
            load, cnt, b = heapq.heappop(heap)
            if cnt < P:
                break
        bin_of[n] = b
        slot_of[n] = cnt
        heapq.heappush(heap, (load + int(deg_in[n]), cnt + 1, b))
    loads = np.bincount(bin_of, weights=deg_in, minlength=BINS).astype(np.int64)
    return bin_of, slot_of, loads


def host_prep(cfg, inputs):
    N, E = cfg.N, cfg.E
    BINS, BPC, NSHARD = cfg.BINS, cfg.BPC, cfg.NSHARD

    x = np.asarray(inputs["x"], np.float32)
    ei = np.asarray(inputs["edge_index"])
    src_o = ei[0].astype(np.int64)
    dst_o = ei[1].astype(np.int64)
    ea = np.asarray(inputs["edge_attr"], np.float32)
    ndeg = np.asarray(inputs["node_degrees"]).astype(np.int64)
    llr = np.asarray(inputs["llr_features"], np.float32).reshape(N)
    vmask = np.asarray(inputs["var_node_mask"]).astype(np.float32).reshape(N)

    deg_in = np.bincount(dst_o, minlength=N).astype(np.int64)
    target = -(-E // BINS)
    bin_of, slot_of, loads = _balance_bins(deg_in, N, BINS, target)
    max_load = int(loads.max())
    S = max(1, -(-max_load // P))
    cfg = Cfg(N, E, S)
    SLOTS = cfg.SLOTS
    CH, WPC, CPW = cfg.CH, cfg.WPC, cfg.CPW

    # permuted node id: node o sits at (bin, slot)
    o2p = bin_of * P + slot_of
    p2o = np.argsort(o2p)          # p2o[pid] = original id

    # x_w DRAM table row of permuted node n: n = slab*512 + t*128 + p is
    # stored at row slab*512 + p*4 + t (matches contiguous slab stores)
    n_ids = np.arange(N)
    n_slab = n_ids // 512
    n_t = (n_ids % 512) // P
    n_p = n_ids % P
    row_of_node = n_slab * 512 + n_p * 4 + n_t

    # --- edge arrays grouped by destination bin ---------------------------
    src_p = o2p[src_o]
    dst_pid = o2p[dst_o]
    ebin = dst_pid >> 7
    eslot = dst_pid & 127

    eorder = np.argsort(ebin, kind="stable")
    ebin_s = ebin[eorder]
    starts = np.zeros(BINS + 1, np.int64)
    np.cumsum(np.bincount(ebin_s, minlength=BINS), out=starts[1:])
    rank = np.arange(E) - starts[ebin_s]
    q = ebin_s * SLOTS + rank                 # position in padded layout

    esrc = np.zeros(BINS * SLOTS, np.int64)   # permuted src node id
    eslot_f = np.full(BINS * SLOTS, float(P), np.float32)   # pad slot = P
    eattr = np.zeros((BINS * SLOTS, 8), np.float32)
    esrc[q] = src_p[eorder]
    eslot_f[q] = eslot[eorder].astype(np.float32)
    eattr[q] = ea[eorder]

    # gather row index per edge slot (into the shuffled x_w table layout).
    # Pad slots point at the last table row (positive int16 after re-basing),
    # and each bin's slots are stably partitioned so that positive-row slots
    # come last: the transpose-gather drops trailing NEGATIVE indices, so the
    # final index of every per-window gather op must be non-negative.
    egrow_f = np.full(BINS * SLOTS, N - 1, np.int64)
    filled = np.zeros(BINS * SLOTS, bool)
    filled[q] = True
    egrow_f[q] = row_of_node[src_p[eorder]]
    eg2 = egrow_f.reshape(BINS, SLOTS)
    es2 = eslot_f.reshape(BINS, SLOTS)
    ea2 = eattr.reshape(BINS, SLOTS, 8)
    order2 = np.argsort(eg2 >= N // 2, axis=1, kind="stable")
    eg2 = np.take_along_axis(eg2, order2, axis=1)
    es2 = np.take_along_axis(es2, order2, axis=1)
    ea2 = np.take_along_axis(ea2, order2[:, :, None], axis=1)
    eslot_f = es2.reshape(-1)
    eattr = ea2.reshape(-1, 8)
    egrow = eg2.reshape(BINS, S, P)                   # [win_glob, j, p]

    # per-core transpose-gather indices: int16 = table_row - N/2 (sign trick
    # extends the addressable range to 65536 rows).  Position i = col*128 + e;
    # the CPW cols of window w are [own nodes, edge subtile 0..S-1].
    NPC = WPC * CPW * P                           # idx positions per chunk
    idx_g = np.zeros((NCORES, CH, P, NPC // 16), np.int16)
    half = N // 2
    for c in range(NCORES):
        for ch in range(CH):
            unw = np.zeros(NPC, np.int64)
            for wdx in range(WPC):
                wg = c * BPC + ch * WPC + wdx     # global bin
                base = wdx * CPW * P
                own_nodes = wg * P + np.arange(P) # permuted ids of own bin
                unw[base:base + P] = row_of_node[own_nodes]
                for j in range(S):
                    unw[base + (1 + j) * P:base + (2 + j) * P] = egrow[wg, j]
            w16 = (unw - half).astype(np.int16).reshape(NPC // 16, 16).T
            idx_g[c, ch] = np.tile(w16, (8, 1))

    eslot_r = eslot_f.reshape(NCORES, BPC * S, P)
    dst_col = eslot_r.transpose(0, 2, 1).copy()               # [c, p, col]
    eattr_r = eattr.reshape(NCORES, BPC * S, P, 8)
    ea_t = eattr_r.transpose(0, 3, 1, 2).reshape(
        NCORES, 8, BPC * S * P).astype(BF)                    # [c, 8, col*p]

    # --- node arrays (full, replicated) -----------------------------------
    xp = x[p2o]                                              # [N, HID]
    x_t_full = np.ascontiguousarray(xp.T.astype(BF))         # [128, N]
    # interleaved rows: [slab, p, t, f], node n = slab*512 + t*128 + p
    xr4 = np.ascontiguousarray(
        xp.reshape(cfg.NSLAB, 4, P, P).transpose(0, 2, 1, 3).astype(BF))
    # llr per node: [p, slab, t]
    llr4 = np.ascontiguousarray(
        llr[p2o].reshape(cfg.NSLAB, 4, P).transpose(2, 0, 1).astype(BF))
    # mask: [p, slab, t]
    m_all = np.ascontiguousarray(
        vmask[p2o].reshape(cfg.NSLAB, 4, P).transpose(2, 0, 1).astype(BF))

    # degree gather indices (int16), wrap order, replicated to 128 parts
    degc = np.clip(ndeg, 0, 99)[p2o].reshape(NCORES, NSHARD).astype(np.int16)
    deg_wrap = degc.reshape(NCORES, NSHARD // 16, 16).transpose(0, 2, 1)
    deg_rep = np.ascontiguousarray(np.tile(deg_wrap, (1, 8, 1)))  # [c,128,S]

    # --- weights -----------------------------------------------------------
    w = {k: np.asarray(v, np.float32) for k, v in inputs.items()
         if k not in ("x", "edge_index", "edge_attr", "node_degrees",
                      "llr_features", "var_node_mask")}
    att = w["att"]                                           # [4,128]

    def bcast_row(v, reps):                                  # [P, reps*128]
        return np.ascontiguousarray(
            np.broadcast_to(np.tile(v, reps)[None, :], (P, reps * P)))

    flags = {
        "gf1": bool(np.allclose(w["g_f"], 1.0)),
        "bef0": bool(np.allclose(w["be_f"], 0.0)),
        "gg1": bool(np.allclose(w["g_g"], 1.0)),
        "beg0": bool(np.allclose(w["be_g"], 0.0)),
        "bg20": bool(np.allclose(w["b_g2"], 0.0)),
        "go1": bool(np.allclose(w["g_o"], 1.0)),
        "bo0": bool(np.allclose(w["b_o"], 0.0)),
    }

    consts = {
        "c_Wfx": w["W_f"][:P].astype(BF),                    # [128,128]
        "c_wfl4": np.ascontiguousarray(np.broadcast_to(
            np.tile(w["W_f"][P], 4)[None, :], (P, 512)).astype(BF)),
        "c_bfc": np.ascontiguousarray(
            w["b_f"].reshape(P, 1).astype(np.float32)),
        "c_eps": np.full((P, 1), LN_EPS, np.float32),
        "c_Wl": w["W_l"].astype(BF),                         # [128,512]
        "c_Wr": w["W_r"].astype(BF),
        "c_We": w["W_e"].astype(BF),                         # [8,512]
        "c_attb": np.ascontiguousarray(
            np.broadcast_to(att.reshape(1, 512), (P, 512)).astype(BF)),
        "c_iota": np.ascontiguousarray(
            np.broadcast_to(np.arange(P, dtype=np.float32)[None, :], (P, P))),
        "c_ident": np.eye(P, dtype=BF),
        "c_Wg1x": w["W_g1"][:P].astype(BF),                  # [128,128]
        "c_Wg2": w["W_g2"].astype(BF),
        "c_Td": (w["deg_emb"] @ w["W_g1"][P:P + 16]
                 + w["b_g1"][None, :]).astype(np.float32),   # [100,128]
    }
    if not flags["gf1"]:
        consts["c_gf4"] = bcast_row(w["g_f"], 4).astype(BF)
    if not flags["bef0"]:
        consts["c_bef4"] = bcast_row(w["be_f"], 4).astype(BF)
    if not flags["gg1"]:
        consts["c_gg4"] = bcast_row(w["g_g"], 4).astype(BF)
    if not flags["beg0"]:
        consts["c_beg4"] = bcast_row(w["be_g"], 4).astype(BF)
    if not flags["bg20"]:
        consts["c_bg24"] = bcast_row(w["b_g2"], 4).astype(np.float32)
    if not flags["go1"]:
        consts["c_go4"] = bcast_row(w["g_o"], 4).astype(np.float32)
    if not flags["bo0"]:
        consts["c_bo4"] = bcast_row(w["b_o"], 4).astype(np.float32)
    consts = {k: np.ascontiguousarray(v) for k, v in consts.items()}

    in_maps = []
    for k in range(NCORES):
        m = {
            "x_t": x_t_full, "xr4": xr4, "llr4": llr4, "m_all": m_all,
            "idx_g": np.ascontiguousarray(idx_g[k]),
            "dst_col": np.ascontiguousarray(dst_col[k]),
            "ea_t": np.ascontiguousarray(ea_t[k]),
            "deg_w": deg_rep[k],
        }
        m.update(consts)
        in_maps.append(m)
    return cfg, in_maps, p2o, flags


# ----------------------------------------------------------------------------
# Device kernel
# ----------------------------------------------------------------------------

def build_kernel(cfg, flags):
    PH = int(os.environ.get("GNN_PH", "3"))
    N, S = cfg.N, cfg.S_SUB
    BPC, NSHARD = cfg.BPC, cfg.NSHARD
    NSLAB, CH, WPC, CPW = cfg.NSLAB, cfg.CH, cfg.WPC, cfg.CPW
    SH_SLAB = NSHARD // 512                      # output slabs per core (16)
    COLS = BPC * S                               # edge subtile columns (256)

    nc = bacc.Bacc("TRN2", target_bir_lowering=False, debug=False,
                   num_devices=NCORES)

    # ---- I/O ----
    d_xt = nc.dram_tensor("x_t", [P, N], BF16, kind="ExternalInput")
    d_xr4 = nc.dram_tensor("xr4", [NSLAB, P, 4, P], BF16, kind="ExternalInput")
    d_llr = nc.dram_tensor("llr4", [P, NSLAB, 4], BF16, kind="ExternalInput")
    d_m = nc.dram_tensor("m_all", [P, NSLAB, 4], BF16, kind="ExternalInput")
    d_idx = nc.dram_tensor("idx_g", [CH, P, WPC * CPW * 8], I16,
                           kind="ExternalInput")
    d_dst = nc.dram_tensor("dst_col", [P, COLS], F32, kind="ExternalInput")
    d_eat = nc.dram_tensor("ea_t", [8, COLS * P], BF16, kind="ExternalInput")
    d_deg = nc.dram_tensor("deg_w", [P, NSHARD // 16], I16,
                           kind="ExternalInput")
    d_out = nc.dram_tensor("y", [SH_SLAB, P, 4, P], F32, kind="ExternalOutput")

    cshape = {
        "c_Wfx": ([P, P], BF16), "c_wfl4": ([P, 512], BF16),
        "c_bfc": ([P, 1], F32), "c_eps": ([P, 1], F32),
        "c_Wl": ([P, 512], BF16), "c_Wr": ([P, 512], BF16),
        "c_We": ([8, 512], BF16), "c_attb": ([P, 512], BF16),
        "c_iota": ([P, P], F32), "c_ident": ([P, P], BF16),
        "c_Wg1x": ([P, P], BF16), "c_Wg2": ([P, P], BF16),
        "c_Td": ([100, P], F32),
    }
    for nm, fl, dt in (("c_gf4", "gf1", BF16), ("c_bef4", "bef0", BF16),
                       ("c_gg4", "gg1", BF16), ("c_beg4", "beg0", BF16),
                       ("c_bg24", "bg20", F32), ("c_go4", "go1", F32),
                       ("c_bo4", "bo0", F32)):
        if not flags[fl]:
            cshape[nm] = ([P, 512], dt)
    d_c = {k: nc.dram_tensor(k, sh, dt, kind="ExternalInput")
           for k, (sh, dt) in cshape.items()}

    # gather table with a shadow copy of the first half appended: the
    # transpose-gather reads rows [0,N) via int16 idx relative to base N/2,
    # but its declared AP is rows [N/2, 2N) -- the shadow writes make every
    # phase-1 store overlap that range so the dep tracker orders them.
    d_xw = nc.dram_tensor("xw_tab", [2 * N, P], BF16)
    d_td = nc.dram_tensor("td_tab", [100, P], F32)

    with tile.TileContext(nc) as tc:
        with (
            tc.tile_pool(name="const", bufs=1) as cpool,
            tc.tile_pool(name="resid", bufs=1) as rpool,
        ):
            C = {}
            for k, (sh, dt) in cshape.items():
                C[k] = cpool.tile(sh, dt, tag=k, name=f"const_{k}")
                nc.sync.dma_start(out=C[k][:], in_=d_c[k].ap())
            nc.sync.dma_start(out=d_td.ap(), in_=C["c_Td"][:])

            llr_s = rpool.tile([P, NSLAB, 4], BF16, tag="llr")
            nc.sync.dma_start(out=llr_s[:], in_=d_llr.ap())
            m_s = rpool.tile([P, NSLAB, 4], BF16, tag="m")
            nc.sync.dma_start(out=m_s[:], in_=d_m.ap())
            dst_t = rpool.tile([P, COLS], F32, tag="dst")
            nc.sync.dma_start(out=dst_t[:], in_=d_dst.ap())
            eat_t = rpool.tile([8, COLS * P], BF16, tag="eat")
            nc.sync.dma_start(out=eat_t[:], in_=d_eat.ap())
            deg_t = rpool.tile([P, NSHARD // 16], I16, tag="deg")
            nc.sync.dma_start(out=deg_t[:], in_=d_deg.ap())

            v2c_nm = rpool.tile([P, BPC, P], BF16, tag="v2c")  # [p, w, f]
            dterm = rpool.tile([P, BPC, P], F32, tag="dterm")

            # degree-embedding term for all own nodes (1024-idx chunks --
            # a single instruction's descriptors must fit the SWDGE ring)
            for g in range(NSHARD // 1024):
                nc.gpsimd.dma_gather(
                    out_ap=dterm[:, g * 8:(g + 1) * 8, :], in_ap=d_td.ap(),
                    idxs_ap=deg_t[:, g * 64:(g + 1) * 64],
                    num_idxs=1024, num_idxs_reg=1024, elem_size=P,
                    transpose=False)

            # ================= Phase 1: LLR fusion (replicated) =============
            with (
                tc.tile_pool(name="p1mm", bufs=2, space="PSUM") as pp1,
                tc.tile_pool(name="p1tr", bufs=2, space="PSUM") as pp1t,
                tc.tile_pool(name="p1in", bufs=3) as sb1i,
                tc.tile_pool(name="p1wk", bufs=2) as sb1,
            ):
                for s in range(NSLAB):
                    ns = slice(s * 512, (s + 1) * 512)
                    xt_sl = sb1i.tile([P, 512], BF16, tag="xt")
                    nc.sync.dma_start(out=xt_sl[:], in_=d_xt.ap()[:, ns])
                    py = pp1.tile([P, 512], F32, tag="y")
                    nc.tensor.matmul(py[:], C["c_Wfx"][:], xt_sl[:],
                                     start=True, stop=True)
                    ytT = sb1.tile([P, 512], BF16, tag="ytT")
                    nc.scalar.activation(ytT[:], py[:], AF.Identity,
                                         bias=C["c_bfc"][:])
                    ptq = pp1t.tile([P, 512], BF16, tag="tr")
                    for t in range(4):
                        qs = slice(t * P, (t + 1) * P)
                        nc.tensor.transpose(ptq[:, qs], ytT[:, qs],
                                            C["c_ident"][:])
                    wl4 = sb1.tile([P, 4, P], BF16, tag="wl4")
                    nc.vector.tensor_tensor(
                        out=wl4[:],
                        in0=C["c_wfl4"][:].rearrange("p (t f) -> p t f", t=4),
                        in1=llr_s[:, s, :].rearrange("p (t o) -> p t o", o=1)
                            .to_broadcast([P, 4, P]), op=OP.mult)
                    yr = sb1.tile([P, 4, P], BF16, tag="yr")
                    nc.vector.tensor_tensor(
                        out=yr[:], in0=ptq[:].rearrange("p (t f) -> p t f", t=4),
                        in1=wl4[:], op=OP.add)
                    bst = sb1.tile([P, 4, 6], F32, tag="bst")
                    mv = sb1.tile([P, 4, 2], F32, tag="mv")
                    for t in range(4):
                        nc.vector.bn_stats(bst[:, t, :], yr[:, t, :])
                        nc.vector.bn_aggr(mv[:, t, :], bst[:, t, :])
                    sd4 = sb1.tile([P, 4], F32, tag="sd4")
                    nc.scalar.activation(sd4[:], mv[:, :, 1], AF.Sqrt,
                                         bias=C["c_eps"][:])
                    iv4 = sb1.tile([P, 4], F32, tag="iv4")
                    nc.vector.reciprocal(iv4[:], sd4[:])
                    nm4 = sb1.tile([P, 4], F32, tag="nm4")
                    nc.vector.scalar_tensor_tensor(
                        out=nm4[:], in0=mv[:, :, 0], scalar=-1.0, in1=iv4[:],
                        op0=OP.mult, op1=OP.mult)
                    t1 = sb1.tile([P, 4, P], BF16, tag="t1")
                    nc.vector.tensor_tensor(
                        out=t1[:], in0=yr[:],
                        in1=iv4[:].rearrange("p (t o) -> p t o", o=1)
                            .to_broadcast([P, 4, P]), op=OP.mult)
                    t2 = sb1.tile([P, 4, P], BF16, tag="t2")
                    nc.vector.tensor_tensor(
                        out=t2[:], in0=t1[:],
                        in1=nm4[:].rearrange("p (t o) -> p t o", o=1)
                            .to_broadcast([P, 4, P]), op=OP.add)
                    zz = t2
                    if not flags["gf1"]:
                        zg = sb1.tile([P, 4, P], BF16, tag="zg")
                        nc.vector.tensor_tensor(
                            out=zg[:], in0=zz[:],
                            in1=C["c_gf4"][:].rearrange(
                                "p (t f) -> p t f", t=4), op=OP.mult)
                        zz = zg
                    if not flags["bef0"]:
                        zb = sb1.tile([P, 4, P], BF16, tag="zb")
                        nc.vector.tensor_tensor(
                            out=zb[:], in0=zz[:],
                            in1=C["c_bef4"][:].rearrange(
                                "p (t f) -> p t f", t=4), op=OP.add)
                        zz = zb
                    fu = sb1.tile([P, 4, P], BF16, tag="fu")
                    nc.scalar.activation(fu[:], zz[:], AF.Relu)
                    xr_sl = sb1i.tile([P, 4, P], BF16, tag="xr")
                    nc.scalar.dma_start(out=xr_sl[:], in_=d_xr4.ap()[s])
                    d1 = sb1.tile([P, 4, P], BF16, tag="d1")
                    nc.vector.tensor_tensor(out=d1[:], in0=fu[:], in1=xr_sl[:],
                                            op=OP.subtract)
                    dm = sb1.tile([P, 4, P], BF16, tag="dm")
                    nc.vector.tensor_tensor(
                        out=dm[:], in0=d1[:],
                        in1=m_s[:, s, :].rearrange("p (t o) -> p t o", o=1)
                            .to_broadcast([P, 4, P]), op=OP.mult)
                    xw_sl = sb1.tile([P, 4, P], BF16, tag="xw")
                    nc.vector.tensor_tensor(out=xw_sl[:], in0=dm[:],
                                            in1=xr_sl[:], op=OP.add)
                    nc.sync.dma_start(
                        out=d_xw.ap()[ns, :].rearrange(
                            "(pp t) f -> pp t f", t=4),
                        in_=xw_sl[:])
                    if s < NSLAB // 2:
                        sh = slice(N + s * 512, N + (s + 1) * 512)
                        nc.scalar.dma_start(
                            out=d_xw.ap()[sh, :].rearrange(
                                "(pp t) f -> pp t f", t=4),
                            in_=xw_sl[:])
                    if PH == 1 and s < SH_SLAB:
                        dbg = sb1.tile([P, 4, P], F32, tag="dbg")
                        nc.vector.tensor_copy(out=dbg[:], in_=xw_sl[:])
                        nc.scalar.dma_start(out=d_out.ap()[s], in_=dbg[:])

            # ================= Phase 2: edges ===============================
            with (
                tc.tile_pool(name="pz", bufs=2, space="PSUM") as ppz,
                tc.tile_pool(name="pxl", bufs=2, space="PSUM") as ppxl,
                tc.tile_pool(name="po4", bufs=1, space="PSUM") as ppo4,
                tc.tile_pool(name="psm", bufs=1, space="PSUM") as ppsm,
                tc.tile_pool(name="ptr", bufs=2, space="PSUM") as pptr,
                tc.tile_pool(name="e_in", bufs=2) as ein,
                tc.tile_pool(name="e_wk", bufs=3) as ewk,
                tc.tile_pool(name="e_w2", bufs=2) as ewk2,
            ):
                NPC = WPC * CPW * P
                NPW = CPW * P                      # idx positions per window
                for ch in range(CH if PH >= 2 else 0):
                    idx_t = ein.tile([P, NPC // 16], I16, tag="idx")
                    nc.sync.dma_start(out=idx_t[:], in_=d_idx.ap()[ch])
                    # feature-major gather: column i holds x_w of idx i
                    xg_t = ein.tile([P, 1, NPC], BF16, tag="xg")
                    for wdx in range(WPC):
                        nc.gpsimd.dma_gather(
                            out_ap=xg_t[:, :, wdx * NPW:(wdx + 1) * NPW],
                            in_ap=d_xw.ap()[N // 2:2 * N, :],
                            idxs_ap=idx_t[:, wdx * (NPW // 16):
                                          (wdx + 1) * (NPW // 16)],
                            num_idxs=NPW, num_idxs_reg=NPW,
                            elem_size=P, transpose=True)

                    for wdx in range(WPC):
                        win = ch * WPC + wdx
                        base = wdx * CPW * P
                        # xr tile for this window's own (dst) nodes
                        xwT = xg_t[:, 0, base:base + P]
                        pxr = ppz.tile([P, 512], F32, tag="z")
                        nc.tensor.matmul(pxr[:], xwT, C["c_Wr"][:],
                                         start=True, stop=True)
                        xr_sb = ewk2.tile([P, 512], BF16, tag="xr")
                        nc.vector.tensor_copy(out=xr_sb[:], in_=pxr[:])

                        pden = ppsm.tile([P, 4], F32, tag="sm")
                        po4 = ppo4.tile([P, 512], F32, tag="o4")

                        for j in range(S):
                            st = win * S + j
                            # one-hot S [e, d] and its transpose
                            S_sb = ewk.tile([P, P], BF16, tag="S")
                            nc.vector.tensor_tensor(
                                out=S_sb[:],
                                in0=dst_t[:, st:st + 1].to_broadcast([P, P]),
                                in1=C["c_iota"][:], op=OP.is_equal)
                            pts = pptr.tile([P, P], BF16, tag="tr")
                            nc.tensor.transpose(pts[:], S_sb[:],
                                                C["c_ident"][:])
                            st_sb = ewk.tile([P, P], BF16, tag="st")
                            nc.scalar.activation(st_sb[:], pts[:], AF.Identity)
                            # gathered x_w[src] columns (feature-major)
                            xgT = xg_t[:, 0, base + (1 + j) * P:
                                       base + (2 + j) * P]

                            ea_sl = eat_t[:, st * P:(st + 1) * P]
                            pz = ppz.tile([P, 512], F32, tag="z")
                            pxl = ppxl.tile([P, 512], F32, tag="xl")
                            nc.tensor.matmul(pz[:], xgT, C["c_Wl"][:],
                                             start=True, stop=False)
                            nc.tensor.matmul(pxl[:], xgT, C["c_Wl"][:],
                                             start=True, stop=True)
                            nc.tensor.matmul(pz[:], st_sb[:], xr_sb[:],
                                             start=False, stop=False)
                            nc.tensor.matmul(pz[:], ea_sl, C["c_We"][:],
                                             start=False, stop=True)

                            # leaky = 0.2*z + 0.8*relu(z)
                            r_sb = ewk.tile([P, 512], BF16, tag="r")
                            nc.scalar.activation(r_sb[:], pz[:], AF.Relu,
                                                 scale=0.8)
                            lk = ewk.tile([P, 512], BF16, tag="lk")
                            nc.vector.scalar_tensor_tensor(
                                out=lk[:], in0=pz[:], scalar=0.2, in1=r_sb[:],
                                op0=OP.mult, op1=OP.add)
                            # alpha[e,h] = sum_c lk*att
                            zat = ewk.tile([P, 512], BF16, tag="zat")
                            nc.vector.tensor_tensor(out=zat[:], in0=lk[:],
                                                    in1=C["c_attb"][:],
                                                    op=OP.mult)
                            alpha = ewk.tile([P, 4], F32, tag="alpha")
                            nc.vector.reduce_sum(
                                out=alpha[:],
                                in_=zat[:].rearrange("p (h c) -> p h c", h=4),
                                axis=AX.X)
                            au = ewk.tile([P, 4], BF16, tag="au")
                            nc.scalar.activation(au[:], alpha[:], AF.Exp)
                            nc.tensor.matmul(pden[:], S_sb[:], au[:],
                                             start=(j == 0), stop=(j == S - 1))
                            # xl scaled by per-edge attention (per head)
                            xla = ewk.tile([P, 4, P], BF16, tag="xla")
                            nc.vector.tensor_tensor(
                                out=xla[:],
                                in0=pxl[:].rearrange("p (h f) -> p h f", h=4),
                                in1=au[:].rearrange("p (h o) -> p h o", o=1)
                                    .to_broadcast([P, 4, P]), op=OP.mult)
                            nc.tensor.matmul(
                                po4[:], S_sb[:],
                                xla[:].rearrange("p h f -> p (h f)"),
                                start=(j == 0), stop=(j == S - 1))
                        # normalize + head mean -> v2c (node-major)
                        dv = ewk.tile([P, 4], F32, tag="dv")
                        nc.vector.tensor_scalar(out=dv[:], in0=pden[:],
                                                scalar1=SM_EPS, scalar2=None,
                                                op0=OP.add)
                        iv = ewk.tile([P, 4], F32, tag="iv")
                        nc.vector.reciprocal(iv[:], dv[:])
                        nc.vector.tensor_scalar(out=iv[:], in0=iv[:],
                                                scalar1=0.25, scalar2=None,
                                                op0=OP.mult)
                        vsl = v2c_nm[:, win, :]
                        nc.vector.tensor_scalar(
                            out=vsl, in0=po4[:, 0:P], scalar1=iv[:, 0:1],
                            scalar2=None, op0=OP.mult)
                        for h in range(1, 4):
                            hs = slice(h * P, (h + 1) * P)
                            nc.vector.scalar_tensor_tensor(
                                out=vsl, in0=po4[:, hs], scalar=iv[:, h:h + 1],
                                in1=vsl, op0=OP.mult, op1=OP.add)

            # ================= Phase 3: degree gate + final LN ==============
            with (
                tc.tile_pool(name="p3a", bufs=2, space="PSUM") as pp3,
                tc.tile_pool(name="p3t", bufs=2, space="PSUM") as pp3t,
                tc.tile_pool(name="g_wk", bufs=2) as gwk,
            ):
                for sl in range(SH_SLAB if PH >= 3 else 0):
                    ws = slice(sl * 4, sl * 4 + 4)
                    # h = v2c @ Wg1x + dterm (deg-emb term incl. b_g1)
                    ph = pp3.tile([P, 512], F32, tag="h")
                    for t in range(4):
                        win = sl * 4 + t
                        ptv = pp3t.tile([P, P], BF16, tag="t")
                        nc.tensor.transpose(ptv[:], v2c_nm[:, win, :],
                                            C["c_ident"][:])
                        v2cT = gwk.tile([P, P], BF16, tag="v2cT")
                        nc.scalar.activation(v2cT[:], ptv[:], AF.Identity)
                        nc.tensor.matmul(ph[:, t * P:(t + 1) * P], v2cT[:],
                                         C["c_Wg1x"][:], start=True, stop=True)
                    h_sb = gwk.tile([P, 4, P], BF16, tag="h_sb")
                    nc.vector.tensor_tensor(
                        out=h_sb[:],
                        in0=ph[:].rearrange("p (t f) -> p t f", t=4),
                        in1=dterm[:, ws, :], op=OP.add)
                    bst = gwk.tile([P, 4, 6], F32, tag="bst")
                    mv = gwk.tile([P, 4, 2], F32, tag="mv")
                    for t in range(4):
                        nc.vector.bn_stats(bst[:, t, :], h_sb[:, t, :])
                        nc.vector.bn_aggr(mv[:, t, :], bst[:, t, :])
                    sd4 = gwk.tile([P, 4], F32, tag="sd4")
                    nc.scalar.activation(sd4[:], mv[:, :, 1], AF.Sqrt,
                                         bias=C["c_eps"][:])
                    iv4 = gwk.tile([P, 4], F32, tag="iv4")
                    nc.vector.reciprocal(iv4[:], sd4[:])
                    nm4 = gwk.tile([P, 4], F32, tag="nm4")
                    nc.vector.scalar_tensor_tensor(
                        out=nm4[:], in0=mv[:, :, 0], scalar=-1.0, in1=iv4[:],
                        op0=OP.mult, op1=OP.mult)
                    t1 = gwk.tile([P, 4, P], BF16, tag="t1")
                    nc.vector.tensor_tensor(
                        out=t1[:], in0=h_sb[:],
                        in1=iv4[:].rearrange("p (t o) -> p t o", o=1)
                            .to_broadcast([P, 4, P]), op=OP.mult)
                    t2 = gwk.tile([P, 4, P], BF16, tag="t2")
                    nc.vector.tensor_tensor(
                        out=t2[:], in0=t1[:],
                        in1=nm4[:].rearrange("p (t o) -> p t o", o=1)
                            .to_broadcast([P, 4, P]), op=OP.add)
                    zz = t2
                    if not flags["gg1"]:
                        zg = gwk.tile([P, 4, P], BF16, tag="zg")
                        nc.vector.tensor_tensor(
                            out=zg[:], in0=zz[:],
                            in1=C["c_gg4"][:].rearrange(
                                "p (t f) -> p t f", t=4), op=OP.mult)
                        zz = zg
                    if not flags["beg0"]:
                        zb = gwk.tile([P, 4, P], BF16, tag="zb")
                        nc.vector.tensor_tensor(
                            out=zb[:], in0=zz[:],
                            in1=C["c_beg4"][:].rearrange(
                                "p (t f) -> p t f", t=4), op=OP.add)
                        zz = zb
                    h2 = gwk.tile([P, 4, P], BF16, tag="h2")
                    nc.scalar.activation(h2[:], zz[:], AF.Relu)
                    # gate = sigmoid(h2 @ Wg2 + b_g2)
                    pg = pp3.tile([P, 512], F32, tag="h")
                    for t in range(4):
                        pth = pp3t.tile([P, P], BF16, tag="t")
                        nc.tensor.transpose(pth[:], h2[:, t, :],
                                            C["c_ident"][:])
                        h2T = gwk.tile([P, P], BF16, tag="h2T")
                        nc.scalar.activation(h2T[:], pth[:], AF.Identity)
                        nc.tensor.matmul(pg[:, t * P:(t + 1) * P], h2T[:],
                                         C["c_Wg2"][:], start=True, stop=True)
                    gsrc = pg[:]
                    if not flags["bg20"]:
                        gp = gwk.tile([P, 512], F32, tag="gp")
                        nc.vector.tensor_tensor(out=gp[:], in0=pg[:],
                                                in1=C["c_bg24"][:], op=OP.add)
                        gsrc = gp[:]
                    gate = gwk.tile([P, 4, P], BF16, tag="gate")
                    nc.scalar.activation(
                        gate[:], gsrc.rearrange("p (t f) -> p t f", t=4),
                        AF.Sigmoid)
                    p_sb = gwk.tile([P, 4, P], BF16, tag="p_sb")
                    nc.vector.tensor_tensor(out=p_sb[:], in0=v2c_nm[:, ws, :],
                                            in1=gate[:], op=OP.mult)
                    # final LN -> f32 out
                    fbst = gwk.tile([P, 4, 6], F32, tag="fbst")
                    fmv = gwk.tile([P, 4, 2], F32, tag="fmv")
                    for t in range(4):
                        nc.vector.bn_stats(fbst[:, t, :], p_sb[:, t, :])
                        nc.vector.bn_aggr(fmv[:, t, :], fbst[:, t, :])
                    fsd = gwk.tile([P, 4], F32, tag="fsd")
                    nc.scalar.activation(fsd[:], fmv[:, :, 1], AF.Sqrt,
                                         bias=C["c_eps"][:])
                    fiv = gwk.tile([P, 4], F32, tag="fiv")
                    nc.vector.reciprocal(fiv[:], fsd[:])
                    fnm = gwk.tile([P, 4], F32, tag="fnm")
                    nc.vector.scalar_tensor_tensor(
                        out=fnm[:], in0=fmv[:, :, 0], scalar=-1.0, in1=fiv[:],
                        op0=OP.mult, op1=OP.mult)
                    y1 = gwk.tile([P, 4, P], F32, tag="y1")
                    nc.vector.tensor_tensor(
                        out=y1[:], in0=p_sb[:],
                        in1=fiv[:].rearrange("p (t o) -> p t o", o=1)
                            .to_broadcast([P, 4, P]), op=OP.mult)
                    y2 = gwk.tile([P, 4, P], F32, tag="y2")
                    nc.vector.tensor_tensor(
                        out=y2[:], in0=y1[:],
                        in1=fnm[:].rearrange("p (t o) -> p t o", o=1)
                            .to_broadcast([P, 4, P]), op=OP.add)
                    yy = y2
                    if not flags["go1"]:
                        y3 = gwk.tile([P, 4, P], F32, tag="y3")
                        nc.vector.tensor_tensor(
                            out=y3[:], in0=yy[:],
                            in1=C["c_go4"][:].rearrange(
                                "p (t f) -> p t f", t=4), op=OP.mult)
                        yy = y3
                    if not flags["bo0"]:
                        y4 = gwk.tile([P, 4, P], F32, tag="y4")
                        nc.vector.tensor_tensor(
                            out=y4[:], in0=yy[:],
                            in1=C["c_bo4"][:].rearrange(
                                "p (t f) -> p t f", t=4), op=OP.add)
                        yy = y4
                    nc.scalar.dma_start(out=d_out.ap()[sl], in_=yy[:])
                if PH == 2:
                    for sl in range(SH_SLAB):
                        ws = slice(sl * 4, sl * 4 + 4)
                        dbg = gwk.tile([P, 4, P], F32, tag="dbg2")
                        nc.vector.tensor_copy(out=dbg[:], in_=v2c_nm[:, ws, :])
                        nc.scalar.dma_start(out=d_out.ap()[sl], in_=dbg[:])

    nc.compile()
    return nc


# ----------------------------------------------------------------------------
# Entry point
# ----------------------------------------------------------------------------

_CACHE = {}


def _get_kernel(cfg, flags):
    key = (cfg.N, cfg.E, cfg.S_SUB, tuple(sorted(flags.items())))
    if key not in _CACHE:
        _CACHE[key] = build_kernel(cfg, flags)
    return _CACHE[key]


def bench_hw(nc, in_maps, iters=32):
    """Build the sharded PJRT callable once; time repeated executions.

    Output buffers are zero-filled ON DEVICE each iteration (no host
    upload in the timed loop).
    """
    import time
    import jax
    from jax.sharding import Mesh, PartitionSpec, NamedSharding
    from jax.experimental.shard_map import shard_map
    import concourse.mybir as mb
    from concourse import bass2jax as b2j

    b2j.install_neuronx_cc_hook()
    n_cores = len(in_maps)
    partition_name = (nc.partition_id_tensor.name
                      if nc.partition_id_tensor else None)
    in_names, out_names, out_avals, zero_outs = [], [], [], []
    for alloc in nc.m.functions[0].allocations:
        if not isinstance(alloc, mb.MemoryLocationSet):
            continue
        name = alloc.memorylocations[0].name
        if alloc.kind == "ExternalInput":
            if name != partition_name:
                in_names.append(name)
        elif alloc.kind == "ExternalOutput":
            out_names.append(name)
            shape = tuple(alloc.tensor_shape)
            dtype = mb.dt.np(alloc.dtype)
            out_avals.append(jax.core.ShapedArray(shape, dtype))
            zero_outs.append(np.zeros(shape, dtype))
    n_params = len(in_names)
    n_outs = len(out_avals)
    in_names.extend(out_names)
    if partition_name is not None:
        in_names.append(partition_name)
    donate = tuple(range(n_params, n_params + n_outs))

    chain = max(1, int(os.environ.get("GNN_CHAIN", "1")))

    def _body(*args):
        ins = list(args[:n_params])
        outs = list(args[n_params:])
        # chain several executions per dispatch; the (fully overwritten)
        # output operands thread through so XLA cannot CSE the calls
        for _ in range(chain):
            operands = ins + outs
            if partition_name is not None:
                operands.append(b2j.partition_id_tensor())
            outs = list(b2j._bass_exec_p.bind(
                *operands,
                out_avals=tuple(out_avals), in_names=tuple(in_names),
                out_names=tuple(out_names), lowering_input_output_aliases=(),
                sim_require_finite=True, sim_require_nnan=True, nc=nc))
        return tuple(outs)

    devices = jax.devices()[:n_cores]
    mesh = Mesh(np.asarray(devices), ("core",))
    sharded = jax.jit(
        shard_map(_body, mesh=mesh,
                  in_specs=(PartitionSpec("core"),) * (n_params + n_outs),
                  out_specs=(PartitionSpec("core"),) * n_outs,
                  check_rep=False),
        donate_argnums=donate, keep_unused=True)

    concat_in = [
        np.concatenate([np.asarray(in_maps[c][in_names[i]])
                        for c in range(n_cores)], axis=0)
        for i in range(n_params)]
    in_shardings = [NamedSharding(mesh, PartitionSpec("core"))] * n_params
    in_bufs = [jax.device_put(a, s) for a, s in zip(concat_in, in_shardings)]

    import jax.numpy as jnp
    zero_sharding = tuple(
        NamedSharding(mesh, PartitionSpec("core")) for _ in range(n_outs))
    zeros_jit = jax.jit(
        lambda: tuple(
            jnp.zeros((n_cores * z.shape[0], *z.shape[1:]), z.dtype)
            for z in zero_outs),
        out_shardings=zero_sharding)

    def fresh_zeros():
        # one jitted fill per set: separate dispatches yield distinct
        # device buffers (a single batched fill gets CSE'd into one
        # buffer, which the exec's internal donation then invalidates)
        return list(zeros_jit())

    out_arrs = sharded(*in_bufs, *fresh_zeros())
    jax.block_until_ready(out_arrs)
    results = [
        {name: np.asarray(out_arrs[i]).reshape(n_cores, *out_avals[i].shape)[c]
         for i, name in enumerate(out_names)}
        for c in range(n_cores)]

    # pre-create all zero sets so the timed loop only dispatches the NEFF
    zsets = [fresh_zeros() for _ in range(iters)]
    jax.block_until_ready(zsets)

    t0 = time.perf_counter()
    outs = [sharded(*in_bufs, *z) for z in zsets]
    jax.block_until_ready(outs)
    dt = (time.perf_counter() - t0) / (iters * chain)
    return results, dt * 1e9


def kernel(**inputs):
    global LAST_EXEC_NS
    N, E = 65536, 262144
    cfg = Cfg(N, E)
    cfg, in_maps, p2o, flags = host_prep(cfg, inputs)
    nc = _get_kernel(cfg, flags)
    if bool(int(os.environ.get("GNN_BENCH", "1"))):
        results, ns = bench_hw(nc, in_maps,
                               iters=int(os.environ.get("GNN_ITERS", "32")))
        LAST_EXEC_NS = ns
    else:
        res = run_bass_kernel_spmd(nc, in_maps, core_ids=list(range(NCORES)))
        results = res.results
        LAST_EXEC_NS = res.exec_time_ns
    NSHARD = cfg.NSHARD
    y_perm = np.concatenate(
        [results[k]["y"].reshape(NSHARD // 512, P, 4, P)
         .transpose(0, 2, 1, 3).reshape(NSHARD, P)
         for k in range(NCORES)], axis=0)
    y = np.empty_like(y_perm)
    y[p2o] = y_perm
    return y.astype(np.float32)


LAST_EXEC_NS = None


# revision 12
# speedup vs baseline: 1.1933x; 1.1933x over previous
"""Trainium2 Bass kernel for the ExplicitV2C GNN layer (GATv2 message passing).

Strategy (8-core SPMD, no collectives):
  * Host: permute nodes into 512 degree-balanced bins of 128 nodes; group
    edges by destination bin; pad each bin to S subtiles of 128 edges.
    Each core owns 64 bins (8192 dst nodes) and all edges targeting them.
  * Device per core:
      Phase 1 (replicated): LLR fusion (Linear+LN+ReLU+mask) over ALL nodes;
        writes the full bf16 x_w table to core-local DRAM (gather source).
      Phase 2 (edges, sharded): batched indirect-DMA gathers of x_w rows
        (2560 rows per DMA op, including each window's own dst nodes), GATv2
        scores with bf16 matmuls, leaky_relu as 0.2*z + 0.8*relu(z),
        segment softmax + weighted aggregation via one-hot matmuls in PSUM.
      Phase 3 (nodes, sharded): degree gate + final LayerNorm; the degree
        embedding term is fetched with a single dma_gather op.
  * Host: reorder the output shards, undo the node permutation.
"""

import os
import sys

sys.path.insert(0, "/opt/trn_rl_repo")

import numpy as np
import ml_dtypes

import concourse.bass as bass
import concourse.bacc as bacc
import concourse.mybir as mybir
import concourse.tile as tile
from concourse.bass import IndirectOffsetOnAxis
from concourse.bass_utils import run_bass_kernel_spmd

F32 = mybir.dt.float32
BF16 = mybir.dt.bfloat16
I32 = mybir.dt.int32
I16 = mybir.dt.int16
AX = mybir.AxisListType
OP = mybir.AluOpType
AF = mybir.ActivationFunctionType

P = 128
NCORES = 8
LN_EPS = 1e-5
SM_EPS = 1e-16
BF = ml_dtypes.bfloat16


class Cfg:
    def __init__(self, N=65536, E=262144, S_SUB=4):
        self.N, self.E, self.S_SUB = N, E, S_SUB
        self.BINS = N // P                       # node bins total (512)
        self.BPC = self.BINS // NCORES           # windows per core (64)
        self.NSHARD = N // NCORES                # nodes per core (8192)
        self.SLOTS = S_SUB * P                   # edge slots per bin
        self.NSLAB = N // 512                    # phase-1 slabs (128)
        self.CH = 16                             # gather chunks per core
        self.WPC = self.BPC // self.CH           # windows per chunk (4)
        self.CPW = S_SUB + 1                     # gather cols per window


# ----------------------------------------------------------------------------
# Host-side preprocessing
# ----------------------------------------------------------------------------

def _balance_bins(deg_in, N, BINS, target):
    """LPT assignment: nodes by in-degree descending onto the lightest bin
    that still has free slots; every bin gets exactly P nodes."""
    import heapq
    order = np.argsort(-deg_in, kind="stable")
    bin_of = np.empty(N, np.int64)
    slot_of = np.empty(N, np.int64)
    heap = [(0, 0, b) for b in range(BINS)]
    heapq.heapify(heap)
    for n in order:
        while True:
            load, cnt, b = heapq.heappop(heap)
            if cnt < P:
                break
        bin_of[n] = b
        slot_of[n] = cnt
        heapq.heappush(heap, (load + int(deg_in[n]), cnt + 1, b))
    loads = np.bincount(bin_of, weights=deg_in, minlength=BINS).astype(np.int64)
    return bin_of, slot_of, loads


def host_prep(cfg, inputs):
    N, E = cfg.N, cfg.E
    BINS, BPC, NSHARD = cfg.BINS, cfg.BPC, cfg.NSHARD

    x = np.asarray(inputs["x"], np.float32)
    ei = np.asarray(inputs["edge_index"])
    src_o = ei[0].astype(np.int64)
    dst_o = ei[1].astype(np.int64)
    ea = np.asarray(inputs["edge_attr"], np.float32)
    ndeg = np.asarray(inputs["node_degrees"]).astype(np.int64)
    llr = np.asarray(inputs["llr_features"], np.float32).reshape(N)
    vmask = np.asarray(inputs["var_node_mask"]).astype(np.float32).reshape(N)

    deg_in = np.bincount(dst_o, minlength=N).astype(np.int64)
    target = -(-E // BINS)
    bin_of, slot_of, loads = _balance_bins(deg_in, N, BINS, target)
    max_load = int(loads.max())
    S = max(1, -(-max_load // P))
    cfg = Cfg(N, E, S)
    SLOTS = cfg.SLOTS
    CH, WPC, CPW = cfg.CH, cfg.WPC, cfg.CPW

    # permuted node id: node o sits at (bin, slot)
    o2p = bin_of * P + slot_of
    p2o = np.argsort(o2p)          # p2o[pid] = original id

    # x_w DRAM table row of permuted node n: n = slab*512 + t*128 + p is
    # stored at row slab*512 + p*4 + t (matches contiguous slab stores)
    n_ids = np.arange(N)
    n_slab = n_ids // 512
    n_t = (n_ids % 512) // P
    n_p = n_ids % P
    row_of_node = n_slab * 512 + n_p * 4 + n_t

    # --- edge arrays grouped by destination bin ---------------------------
    src_p = o2p[src_o]
    dst_pid = o2p[dst_o]
    ebin = dst_pid >> 7
    eslot = dst_pid & 127

    eorder = np.argsort(ebin, kind="stable")
    ebin_s = ebin[eorder]
    starts = np.zeros(BINS + 1, np.int64)
    np.cumsum(np.bincount(ebin_s, minlength=BINS), out=starts[1:])
    rank = np.arange(E) - starts[ebin_s]
    q = ebin_s * SLOTS + rank                 # position in padded layout

    esrc = np.zeros(BINS * SLOTS, np.int64)   # permuted src node id
    eslot_f = np.full(BINS * SLOTS, float(P), np.float32)   # pad slot = P
    eattr = np.zeros((BINS * SLOTS, 8), np.float32)
    esrc[q] = src_p[eorder]
    eslot_f[q] = eslot[eorder].astype(np.float32)
    eattr[q] = ea[eorder]

    # gather row index per edge slot (into the shuffled x_w table layout).
    # Pad slots point at the last table row (positive int16 after re-basing),
    # and each bin's slots are stably partitioned so that positive-row slots
    # come last: the transpose-gather drops trailing NEGATIVE indices, so the
    # final index of every per-window gather op must be non-negative.
    egrow_f = np.full(BINS * SLOTS, N - 1, np.int64)
    filled = np.zeros(BINS * SLOTS, bool)
    filled[q] = True
    egrow_f[q] = row_of_node[src_p[eorder]]
    eg2 = egrow_f.reshape(BINS, SLOTS)
    es2 = eslot_f.reshape(BINS, SLOTS)
    ea2 = eattr.reshape(BINS, SLOTS, 8)
    order2 = np.argsort(eg2 >= N // 2, axis=1, kind="stable")
    eg2 = np.take_along_axis(eg2, order2, axis=1)
    es2 = np.take_along_axis(es2, order2, axis=1)
    ea2 = np.take_along_axis(ea2, order2[:, :, None], axis=1)
    eslot_f = es2.reshape(-1)
    eattr = ea2.reshape(-1, 8)
    egrow = eg2.reshape(BINS, S, P)                   # [win_glob, j, p]

    # per-core transpose-gather indices: int16 = table_row - N/2 (sign trick
    # extends the addressable range to 65536 rows).  Position i = col*128 + e;
    # the CPW cols of window w are [own nodes, edge subtile 0..S-1].
    NPC = WPC * CPW * P                           # idx positions per chunk
    idx_g = np.zeros((NCORES, CH, P, NPC // 16), np.int16)
    half = N // 2
    for c in range(NCORES):
        for ch in range(CH):
            unw = np.zeros(NPC, np.int64)
            for wdx in range(WPC):
                wg = c * BPC + ch * WPC + wdx     # global bin
                base = wdx * CPW * P
                own_nodes = wg * P + np.arange(P) # permuted ids of own bin
                unw[base:base + P] = row_of_node[own_nodes]
                for j in range(S):
                    unw[base + (1 + j) * P:base + (2 + j) * P] = egrow[wg, j]
            w16 = (unw - half).astype(np.int16).reshape(NPC // 16, 16).T
            idx_g[c, ch] = np.tile(w16, (8, 1))

    eslot_r = eslot_f.reshape(NCORES, BPC * S, P)
    dst_col = eslot_r.transpose(0, 2, 1).copy()               # [c, p, col]
    eattr_r = eattr.reshape(NCORES, BPC * S, P, 8)
    ea_t = eattr_r.transpose(0, 3, 1, 2).reshape(
        NCORES, 8, BPC * S * P).astype(BF)                    # [c, 8, col*p]

    # --- node arrays (full, replicated) -----------------------------------
    xp = x[p2o]                                              # [N, HID]
    x_t_full = np.ascontiguousarray(xp.T.astype(BF))         # [128, N]
    # interleaved rows: [slab, p, t, f], node n = slab*512 + t*128 + p
    xr4 = np.ascontiguousarray(
        xp.reshape(cfg.NSLAB, 4, P, P).transpose(0, 2, 1, 3).astype(BF))
    # llr per node: [p, slab, t]
    llr4 = np.ascontiguousarray(
        llr[p2o].reshape(cfg.NSLAB, 4, P).transpose(2, 0, 1).astype(BF))
    # mask: [p, slab, t]
    m_all = np.ascontiguousarray(
        vmask[p2o].reshape(cfg.NSLAB, 4, P).transpose(2, 0, 1).astype(BF))

    # degree gather indices (int16), wrap order, replicated to 128 parts
    degc = np.clip(ndeg, 0, 99)[p2o].reshape(NCORES, NSHARD).astype(np.int16)
    deg_wrap = degc.reshape(NCORES, NSHARD // 16, 16).transpose(0, 2, 1)
    deg_rep = np.ascontiguousarray(np.tile(deg_wrap, (1, 8, 1)))  # [c,128,S]

    # --- weights -----------------------------------------------------------
    w = {k: np.asarray(v, np.float32) for k, v in inputs.items()
         if k not in ("x", "edge_index", "edge_attr", "node_degrees",
                      "llr_features", "var_node_mask")}
    att = w["att"]                                           # [4,128]

    def bcast_row(v, reps):                                  # [P, reps*128]
        return np.ascontiguousarray(
            np.broadcast_to(np.tile(v, reps)[None, :], (P, reps * P)))

    flags = {
        "gf1": bool(np.allclose(w["g_f"], 1.0)),
        "bef0": bool(np.allclose(w["be_f"], 0.0)),
        "gg1": bool(np.allclose(w["g_g"], 1.0)),
        "beg0": bool(np.allclose(w["be_g"], 0.0)),
        "bg20": bool(np.allclose(w["b_g2"], 0.0)),
        "go1": bool(np.allclose(w["g_o"], 1.0)),
        "bo0": bool(np.allclose(w["b_o"], 0.0)),
    }

    consts = {
        "c_Wfx": w["W_f"][:P].astype(BF),                    # [128,128]
        "c_wfl4": np.ascontiguousarray(np.broadcast_to(
            np.tile(w["W_f"][P], 4)[None, :], (P, 512)).astype(BF)),
        "c_bfc": np.ascontiguousarray(
            w["b_f"].reshape(P, 1).astype(np.float32)),
        "c_eps": np.full((P, 1), LN_EPS, np.float32),
        "c_Wl": w["W_l"].astype(BF),                         # [128,512]
        "c_Wr": w["W_r"].astype(BF),
        "c_We": w["W_e"].astype(BF),                         # [8,512]
        "c_attb": np.ascontiguousarray(
            np.broadcast_to(att.reshape(1, 512), (P, 512)).astype(BF)),
        "c_iota": np.ascontiguousarray(
            np.broadcast_to(np.arange(P, dtype=np.float32)[None, :], (P, P))),
        "c_ident": np.eye(P, dtype=BF),
        "c_Wg1x": w["W_g1"][:P].astype(BF),                  # [128,128]
        "c_Wg2": w["W_g2"].astype(BF),
        "c_Td": (w["deg_emb"] @ w["W_g1"][P:P + 16]
                 + w["b_g1"][None, :]).astype(np.float32),   # [100,128]
    }
    if not flags["gf1"]:
        consts["c_gf4"] = bcast_row(w["g_f"], 4).astype(BF)
    if not flags["bef0"]:
        consts["c_bef4"] = bcast_row(w["be_f"], 4).astype(BF)
    if not flags["gg1"]:
        consts["c_gg4"] = bcast_row(w["g_g"], 4).astype(BF)
    if not flags["beg0"]:
        consts["c_beg4"] = bcast_row(w["be_g"], 4).astype(BF)
    if not flags["bg20"]:
        consts["c_bg24"] = bcast_row(w["b_g2"], 4).astype(np.float32)
    if not flags["go1"]:
        consts["c_go4"] = bcast_row(w["g_o"], 4).astype(np.float32)
    if not flags["bo0"]:
        consts["c_bo4"] = bcast_row(w["b_o"], 4).astype(np.float32)
    consts = {k: np.ascontiguousarray(v) for k, v in consts.items()}

    in_maps = []
    for k in range(NCORES):
        m = {
            "x_t": x_t_full, "xr4": xr4, "llr4": llr4, "m_all": m_all,
            "idx_g": np.ascontiguousarray(idx_g[k]),
            "dst_col": np.ascontiguousarray(dst_col[k]),
            "ea_t": np.ascontiguousarray(ea_t[k]),
            "deg_w": deg_rep[k],
        }
        m.update(consts)
        in_maps.append(m)
    return cfg, in_maps, p2o, flags


# ----------------------------------------------------------------------------
# Device kernel
# ----------------------------------------------------------------------------

def build_kernel(cfg, flags):
    PH = int(os.environ.get("GNN_PH", "3"))
    N, S = cfg.N, cfg.S_SUB
    BPC, NSHARD = cfg.BPC, cfg.NSHARD
    NSLAB, CH, WPC, CPW = cfg.NSLAB, cfg.CH, cfg.WPC, cfg.CPW
    SH_SLAB = NSHARD // 512                      # output slabs per core (16)
    COLS = BPC * S                               # edge subtile columns (256)

    nc = bacc.Bacc("TRN2", target_bir_lowering=False, debug=False,
                   num_devices=NCORES)

    # ---- I/O ----
    d_xt = nc.dram_tensor("x_t", [P, N], BF16, kind="ExternalInput")
    d_xr4 = nc.dram_tensor("xr4", [NSLAB, P, 4, P], BF16, kind="ExternalInput")
    d_llr = nc.dram_tensor("llr4", [P, NSLAB, 4], BF16, kind="ExternalInput")
    d_m = nc.dram_tensor("m_all", [P, NSLAB, 4], BF16, kind="ExternalInput")
    d_idx = nc.dram_tensor("idx_g", [CH, P, WPC * CPW * 8], I16,
                           kind="ExternalInput")
    d_dst = nc.dram_tensor("dst_col", [P, COLS], F32, kind="ExternalInput")
    d_eat = nc.dram_tensor("ea_t", [8, COLS * P], BF16, kind="ExternalInput")
    d_deg = nc.dram_tensor("deg_w", [P, NSHARD // 16], I16,
                           kind="ExternalInput")
    d_out = nc.dram_tensor("y", [SH_SLAB, P, 4, P], F32, kind="ExternalOutput")

    cshape = {
        "c_Wfx": ([P, P], BF16), "c_wfl4": ([P, 512], BF16),
        "c_bfc": ([P, 1], F32), "c_eps": ([P, 1], F32),
        "c_Wl": ([P, 512], BF16), "c_Wr": ([P, 512], BF16),
        "c_We": ([8, 512], BF16), "c_attb": ([P, 512], BF16),
        "c_iota": ([P, P], F32), "c_ident": ([P, P], BF16),
        "c_Wg1x": ([P, P], BF16), "c_Wg2": ([P, P], BF16),
        "c_Td": ([100, P], F32),
    }
    for nm, fl, dt in (("c_gf4", "gf1", BF16), ("c_bef4", "bef0", BF16),
                       ("c_gg4", "gg1", BF16), ("c_beg4", "beg0", BF16),
                       ("c_bg24", "bg20", F32), ("c_go4", "go1", F32),
                       ("c_bo4", "bo0", F32)):
        if not flags[fl]:
            cshape[nm] = ([P, 512], dt)
    d_c = {k: nc.dram_tensor(k, sh, dt, kind="ExternalInput")
           for k, (sh, dt) in cshape.items()}

    # gather table with a shadow copy of the first half appended: the
    # transpose-gather reads rows [0,N) via int16 idx relative to base N/2,
    # but its declared AP is rows [N/2, 2N) -- the shadow writes make every
    # phase-1 store overlap that range so the dep tracker orders them.
    d_xw = nc.dram_tensor("xw_tab", [2 * N, P], BF16)
    d_td = nc.dram_tensor("td_tab", [100, P], F32)

    with tile.TileContext(nc) as tc:
        with (
            tc.tile_pool(name="const", bufs=1) as cpool,
            tc.tile_pool(name="resid", bufs=1) as rpool,
        ):
            C = {}
            for k, (sh, dt) in cshape.items():
                C[k] = cpool.tile(sh, dt, tag=k, name=f"const_{k}")
                nc.sync.dma_start(out=C[k][:], in_=d_c[k].ap())
            nc.sync.dma_start(out=d_td.ap(), in_=C["c_Td"][:])

            llr_s = rpool.tile([P, NSLAB, 4], BF16, tag="llr")
            nc.sync.dma_start(out=llr_s[:], in_=d_llr.ap())
            m_s = rpool.tile([P, NSLAB, 4], BF16, tag="m")
            nc.sync.dma_start(out=m_s[:], in_=d_m.ap())
            dst_t = rpool.tile([P, COLS], F32, tag="dst")
            nc.sync.dma_start(out=dst_t[:], in_=d_dst.ap())
            eat_t = rpool.tile([8, COLS * P], BF16, tag="eat")
            nc.sync.dma_start(out=eat_t[:], in_=d_eat.ap())
            deg_t = rpool.tile([P, NSHARD // 16], I16, tag="deg")
            nc.sync.dma_start(out=deg_t[:], in_=d_deg.ap())

            v2c_nm = rpool.tile([P, BPC, P], BF16, tag="v2c")  # [p, w, f]
            dterm = rpool.tile([P, BPC, P], F32, tag="dterm")

            # degree-embedding term for all own nodes (1024-idx chunks --
            # a single instruction's descriptors must fit the SWDGE ring)
            for g in range(NSHARD // 1024):
                nc.gpsimd.dma_gather(
                    out_ap=dterm[:, g * 8:(g + 1) * 8, :], in_ap=d_td.ap(),
                    idxs_ap=deg_t[:, g * 64:(g + 1) * 64],
                    num_idxs=1024, num_idxs_reg=1024, elem_size=P,
                    transpose=False)

            # ================= Phase 1: LLR fusion (replicated) =============
            with (
                tc.tile_pool(name="p1mm", bufs=2, space="PSUM") as pp1,
                tc.tile_pool(name="p1tr", bufs=2, space="PSUM") as pp1t,
                tc.tile_pool(name="p1in", bufs=3) as sb1i,
                tc.tile_pool(name="p1wk", bufs=2) as sb1,
            ):
                for s in range(NSLAB):
                    ns = slice(s * 512, (s + 1) * 512)
                    xt_sl = sb1i.tile([P, 512], BF16, tag="xt")
                    nc.sync.dma_start(out=xt_sl[:], in_=d_xt.ap()[:, ns])
                    py = pp1.tile([P, 512], F32, tag="y")
                    nc.tensor.matmul(py[:], C["c_Wfx"][:], xt_sl[:],
                                     start=True, stop=True)
                    ytT = sb1.tile([P, 512], BF16, tag="ytT")
                    nc.scalar.activation(ytT[:], py[:], AF.Identity,
                                         bias=C["c_bfc"][:])
                    ptq = pp1t.tile([P, 512], BF16, tag="tr")
                    for t in range(4):
                        qs = slice(t * P, (t + 1) * P)
                        nc.tensor.transpose(ptq[:, qs], ytT[:, qs],
                                            C["c_ident"][:])
                    wl4 = sb1.tile([P, 4, P], BF16, tag="wl4")
                    nc.vector.tensor_tensor(
                        out=wl4[:],
                        in0=C["c_wfl4"][:].rearrange("p (t f) -> p t f", t=4),
                        in1=llr_s[:, s, :].rearrange("p (t o) -> p t o", o=1)
                            .to_broadcast([P, 4, P]), op=OP.mult)
                    yr = sb1.tile([P, 4, P], BF16, tag="yr")
                    nc.vector.tensor_tensor(
                        out=yr[:], in0=ptq[:].rearrange("p (t f) -> p t f", t=4),
                        in1=wl4[:], op=OP.add)
                    bst = sb1.tile([P, 4, 6], F32, tag="bst")
                    mv = sb1.tile([P, 4, 2], F32, tag="mv")
                    for t in range(4):
                        nc.vector.bn_stats(bst[:, t, :], yr[:, t, :])
                        nc.vector.bn_aggr(mv[:, t, :], bst[:, t, :])
                    sd4 = sb1.tile([P, 4], F32, tag="sd4")
                    nc.scalar.activation(sd4[:], mv[:, :, 1], AF.Sqrt,
                                         bias=C["c_eps"][:])
                    iv4 = sb1.tile([P, 4], F32, tag="iv4")
                    nc.vector.reciprocal(iv4[:], sd4[:])
                    nm4 = sb1.tile([P, 4], F32, tag="nm4")
                    nc.vector.scalar_tensor_tensor(
                        out=nm4[:], in0=mv[:, :, 0], scalar=-1.0, in1=iv4[:],
                        op0=OP.mult, op1=OP.mult)
                    t1 = sb1.tile([P, 4, P], BF16, tag="t1")
                    nc.vector.tensor_tensor(
                        out=t1[:], in0=yr[:],
                        in1=iv4[:].rearrange("p (t o) -> p t o", o=1)
                            .to_broadcast([P, 4, P]), op=OP.mult)
                    t2 = sb1.tile([P, 4, P], BF16, tag="t2")
                    nc.vector.tensor_tensor(
                        out=t2[:], in0=t1[:],
                        in1=nm4[:].rearrange("p (t o) -> p t o", o=1)
                            .to_broadcast([P, 4, P]), op=OP.add)
                    zz = t2
                    if not flags["gf1"]:
                        zg = sb1.tile([P, 4, P], BF16, tag="zg")
                        nc.vector.tensor_tensor(
                            out=zg[:], in0=zz[:],
                            in1=C["c_gf4"][:].rearrange(
                                "p (t f) -> p t f", t=4), op=OP.mult)
                        zz = zg
                    if not flags["bef0"]:
                        zb = sb1.tile([P, 4, P], BF16, tag="zb")
                        nc.vector.tensor_tensor(
                            out=zb[:], in0=zz[:],
                            in1=C["c_bef4"][:].rearrange(
                                "p (t f) -> p t f", t=4), op=OP.add)
                        zz = zb
                    fu = sb1.tile([P, 4, P], BF16, tag="fu")
                    nc.scalar.activation(fu[:], zz[:], AF.Relu)
                    xr_sl = sb1i.tile([P, 4, P], BF16, tag="xr")
                    nc.scalar.dma_start(out=xr_sl[:], in_=d_xr4.ap()[s])
                    d1 = sb1.tile([P, 4, P], BF16, tag="d1")
                    nc.vector.tensor_tensor(out=d1[:], in0=fu[:], in1=xr_sl[:],
                                            op=OP.subtract)
                    dm = sb1.tile([P, 4, P], BF16, tag="dm")
                    nc.vector.tensor_tensor(
                        out=dm[:], in0=d1[:],
                        in1=m_s[:, s, :].rearrange("p (t o) -> p t o", o=1)
                            .to_broadcast([P, 4, P]), op=OP.mult)
                    xw_sl = sb1.tile([P, 4, P], BF16, tag="xw")
                    nc.vector.tensor_tensor(out=xw_sl[:], in0=dm[:],
                                            in1=xr_sl[:], op=OP.add)
                    nc.sync.dma_start(
                        out=d_xw.ap()[ns, :].rearrange(
                            "(pp t) f -> pp t f", t=4),
                        in_=xw_sl[:])
                    if s < NSLAB // 2:
                        sh = slice(N + s * 512, N + (s + 1) * 512)
                        nc.scalar.dma_start(
                            out=d_xw.ap()[sh, :].rearrange(
                                "(pp t) f -> pp t f", t=4),
                            in_=xw_sl[:])
                    if PH == 1 and s < SH_SLAB:
                        dbg = sb1.tile([P, 4, P], F32, tag="dbg")
                        nc.vector.tensor_copy(out=dbg[:], in_=xw_sl[:])
                        nc.scalar.dma_start(out=d_out.ap()[s], in_=dbg[:])

            # ================= Phase 2: edges ===============================
            with (
                tc.tile_pool(name="pz", bufs=2, space="PSUM") as ppz,
                tc.tile_pool(name="po4", bufs=2, space="PSUM") as ppo4,
                tc.tile_pool(name="psm", bufs=1, space="PSUM") as ppsm,
                tc.tile_pool(name="ptr", bufs=2, space="PSUM") as pptr,
                tc.tile_pool(name="e_in", bufs=2) as ein,
                tc.tile_pool(name="e_wk", bufs=3) as ewk,
                tc.tile_pool(name="e_w2", bufs=2) as ewk2,
            ):
                NPC = WPC * CPW * P
                NPW = CPW * P                      # idx positions per window
                for ch in range(CH if PH >= 2 else 0):
                    idx_t = ein.tile([P, NPC // 16], I16, tag="idx")
                    nc.sync.dma_start(out=idx_t[:], in_=d_idx.ap()[ch])
                    # feature-major gather: column i holds x_w of idx i
                    xg_t = ein.tile([P, 1, NPC], BF16, tag="xg")
                    for wdx in range(WPC):
                        nc.gpsimd.dma_gather(
                            out_ap=xg_t[:, :, wdx * NPW:(wdx + 1) * NPW],
                            in_ap=d_xw.ap()[N // 2:2 * N, :],
                            idxs_ap=idx_t[:, wdx * (NPW // 16):
                                          (wdx + 1) * (NPW // 16)],
                            num_idxs=NPW, num_idxs_reg=NPW,
                            elem_size=P, transpose=True)

                    for wdx in range(WPC):
                        win = ch * WPC + wdx
                        base = wdx * CPW * P
                        # xr tile for this window's own (dst) nodes
                        xwT = xg_t[:, 0, base:base + P]
                        pxr = ppz.tile([P, 512], F32, tag="z")
                        nc.tensor.matmul(pxr[:], xwT, C["c_Wr"][:],
                                         start=True, stop=True)
                        xr_sb = ewk2.tile([P, 512], BF16, tag="xr")
                        nc.vector.tensor_copy(out=xr_sb[:], in_=pxr[:])

                        pden = ppsm.tile([P, 4], F32, tag="sm")
                        po4 = ppo4.tile([P, 512], F32, tag="o4")

                        for j in range(S):
                            st = win * S + j
                            # one-hot S [e, d] and its transpose
                            S_sb = ewk.tile([P, P], BF16, tag="S")
                            nc.vector.tensor_tensor(
                                out=S_sb[:],
                                in0=dst_t[:, st:st + 1].to_broadcast([P, P]),
                                in1=C["c_iota"][:], op=OP.is_equal)
                            pts = pptr.tile([P, P], BF16, tag="tr")
                            nc.tensor.transpose(pts[:], S_sb[:],
                                                C["c_ident"][:])
                            st_sb = ewk.tile([P, P], BF16, tag="st")
                            nc.scalar.activation(st_sb[:], pts[:], AF.Identity)
                            # gathered x_w[src] columns (feature-major)
                            xgT = xg_t[:, 0, base + (1 + j) * P:
                                       base + (2 + j) * P]

                            ea_sl = eat_t[:, st * P:(st + 1) * P]
                            pz = ppz.tile([P, 512], F32, tag="z")
                            # x_l lands first; copy it out, then keep
                            # accumulating the other z terms onto the bank
                            nc.tensor.matmul(pz[:], xgT, C["c_Wl"][:],
                                             start=True, stop=True)
                            xl_sb = ewk.tile([P, 512], BF16, tag="xl")
                            nc.vector.tensor_copy(out=xl_sb[:], in_=pz[:])
                            nc.tensor.matmul(pz[:], st_sb[:], xr_sb[:],
                                             start=False, stop=False,
                                             skip_group_check=True)
                            nc.tensor.matmul(pz[:], ea_sl, C["c_We"][:],
                                             start=False, stop=True,
                                             skip_group_check=True)

                            # leaky = 0.2*z + 0.8*relu(z)
                            r_sb = ewk.tile([P, 512], BF16, tag="r")
                            nc.scalar.activation(r_sb[:], pz[:], AF.Relu,
                                                 scale=0.8)
                            lk = ewk.tile([P, 512], BF16, tag="lk")
                            nc.vector.scalar_tensor_tensor(
                                out=lk[:], in0=pz[:], scalar=0.2, in1=r_sb[:],
                                op0=OP.mult, op1=OP.add)
                            # alpha[e,h] = sum_c lk*att
                            zat = ewk.tile([P, 512], BF16, tag="zat")
                            nc.vector.tensor_tensor(out=zat[:], in0=lk[:],
                                                    in1=C["c_attb"][:],
                                                    op=OP.mult)
                            alpha = ewk.tile([P, 4], F32, tag="alpha")
                            nc.vector.reduce_sum(
                                out=alpha[:],
                                in_=zat[:].rearrange("p (h c) -> p h c", h=4),
                                axis=AX.X)
                            au = ewk.tile([P, 4], BF16, tag="au")
                            nc.scalar.activation(au[:], alpha[:], AF.Exp)
                            nc.tensor.matmul(pden[:], S_sb[:], au[:],
                                             start=(j == 0), stop=(j == S - 1))
                            # xl scaled by per-edge attention (per head)
                            xla = ewk.tile([P, 4, P], BF16, tag="xla")
                            nc.vector.tensor_tensor(
                                out=xla[:],
                                in0=xl_sb[:].rearrange("p (h f) -> p h f", h=4),
                                in1=au[:].rearrange("p (h o) -> p h o", o=1)
                                    .to_broadcast([P, 4, P]), op=OP.mult)
                            nc.tensor.matmul(
                                po4[:], S_sb[:],
                                xla[:].rearrange("p h f -> p (h f)"),
                                start=(j == 0), stop=(j == S - 1))
                        # normalize + head mean -> v2c (node-major)
                        dv = ewk.tile([P, 4], F32, tag="dv")
                        nc.vector.tensor_scalar(out=dv[:], in0=pden[:],
                                                scalar1=SM_EPS, scalar2=None,
                                                op0=OP.add)
                        iv = ewk.tile([P, 4], F32, tag="iv")
                        nc.vector.reciprocal(iv[:], dv[:])
                        nc.vector.tensor_scalar(out=iv[:], in0=iv[:],
                                                scalar1=0.25, scalar2=None,
                                                op0=OP.mult)
                        vsl = v2c_nm[:, win, :]
                        nc.vector.tensor_scalar(
                            out=vsl, in0=po4[:, 0:P], scalar1=iv[:, 0:1],
                            scalar2=None, op0=OP.mult)
                        for h in range(1, 4):
                            hs = slice(h * P, (h + 1) * P)
                            nc.vector.scalar_tensor_tensor(
                                out=vsl, in0=po4[:, hs], scalar=iv[:, h:h + 1],
                                in1=vsl, op0=OP.mult, op1=OP.add)

            # ================= Phase 3: degree gate + final LN ==============
            with (
                tc.tile_pool(name="p3a", bufs=2, space="PSUM") as pp3,
                tc.tile_pool(name="p3t", bufs=2, space="PSUM") as pp3t,
                tc.tile_pool(name="g_wk", bufs=2) as gwk,
            ):
                for sl in range(SH_SLAB if PH >= 3 else 0):
                    ws = slice(sl * 4, sl * 4 + 4)
                    # h = v2c @ Wg1x + dterm (deg-emb term incl. b_g1)
                    ph = pp3.tile([P, 512], F32, tag="h")
                    for t in range(4):
                        win = sl * 4 + t
                        ptv = pp3t.tile([P, P], BF16, tag="t")
                        nc.tensor.transpose(ptv[:], v2c_nm[:, win, :],
                                            C["c_ident"][:])
                        v2cT = gwk.tile([P, P], BF16, tag="v2cT")
                        nc.scalar.activation(v2cT[:], ptv[:], AF.Identity)
                        nc.tensor.matmul(ph[:, t * P:(t + 1) * P], v2cT[:],
                                         C["c_Wg1x"][:], start=True, stop=True)
                    h_sb = gwk.tile([P, 4, P], BF16, tag="h_sb")
                    nc.vector.tensor_tensor(
                        out=h_sb[:],
                        in0=ph[:].rearrange("p (t f) -> p t f", t=4),
                        in1=dterm[:, ws, :], op=OP.add)
                    bst = gwk.tile([P, 4, 6], F32, tag="bst")
                    mv = gwk.tile([P, 4, 2], F32, tag="mv")
                    for t in range(4):
                        nc.vector.bn_stats(bst[:, t, :], h_sb[:, t, :])
                        nc.vector.bn_aggr(mv[:, t, :], bst[:, t, :])
                    sd4 = gwk.tile([P, 4], F32, tag="sd4")
                    nc.scalar.activation(sd4[:], mv[:, :, 1], AF.Sqrt,
                                         bias=C["c_eps"][:])
                    iv4 = gwk.tile([P, 4], F32, tag="iv4")
                    nc.vector.reciprocal(iv4[:], sd4[:])
                    nm4 = gwk.tile([P, 4], F32, tag="nm4")
                    nc.vector.scalar_tensor_tensor(
                        out=nm4[:], in0=mv[:, :, 0], scalar=-1.0, in1=iv4[:],
                        op0=OP.mult, op1=OP.mult)
                    t1 = gwk.tile([P, 4, P], BF16, tag="t1")
                    nc.vector.tensor_tensor(
                        out=t1[:], in0=h_sb[:],
                        in1=iv4[:].rearrange("p (t o) -> p t o", o=1)
                            .to_broadcast([P, 4, P]), op=OP.mult)
                    t2 = gwk.tile([P, 4, P], BF16, tag="t2")
                    nc.vector.tensor_tensor(
                        out=t2[:], in0=t1[:],
                        in1=nm4[:].rearrange("p (t o) -> p t o", o=1)
                            .to_broadcast([P, 4, P]), op=OP.add)
                    zz = t2
                    if not flags["gg1"]:
                        zg = gwk.tile([P, 4, P], BF16, tag="zg")
                        nc.vector.tensor_tensor(
                            out=zg[:], in0=zz[:],
                            in1=C["c_gg4"][:].rearrange(
                                "p (t f) -> p t f", t=4), op=OP.mult)
                        zz = zg
                    if not flags["beg0"]:
                        zb = gwk.tile([P, 4, P], BF16, tag="zb")
                        nc.vector.tensor_tensor(
                            out=zb[:], in0=zz[:],
                            in1=C["c_beg4"][:].rearrange(
                                "p (t f) -> p t f", t=4), op=OP.add)
                        zz = zb
                    h2 = gwk.tile([P, 4, P], BF16, tag="h2")
                    nc.scalar.activation(h2[:], zz[:], AF.Relu)
                    # gate = sigmoid(h2 @ Wg2 + b_g2)
                    pg = pp3.tile([P, 512], F32, tag="h")
                    for t in range(4):
                        pth = pp3t.tile([P, P], BF16, tag="t")
                        nc.tensor.transpose(pth[:], h2[:, t, :],
                                            C["c_ident"][:])
                        h2T = gwk.tile([P, P], BF16, tag="h2T")
                        nc.scalar.activation(h2T[:], pth[:], AF.Identity)
                        nc.tensor.matmul(pg[:, t * P:(t + 1) * P], h2T[:],
                                         C["c_Wg2"][:], start=True, stop=True)
                    gsrc = pg[:]
                    if not flags["bg20"]:
                        gp = gwk.tile([P, 512], F32, tag="gp")
                        nc.vector.tensor_tensor(out=gp[:], in0=pg[:],
                                                in1=C["c_bg24"][:], op=OP.add)
                        gsrc = gp[:]
                    gate = gwk.tile([P, 4, P], BF16, tag="gate")
                    nc.scalar.activation(
                        gate[:], gsrc.rearrange("p (t f) -> p t f", t=4),
                        AF.Sigmoid)
                    p_sb = gwk.tile([P, 4, P], BF16, tag="p_sb")
                    nc.vector.tensor_tensor(out=p_sb[:], in0=v2c_nm[:, ws, :],
                                            in1=gate[:], op=OP.mult)
                    # final LN -> f32 out
                    fbst = gwk.tile([P, 4, 6], F32, tag="fbst")
                    fmv = gwk.tile([P, 4, 2], F32, tag="fmv")
                    for t in range(4):
                        nc.vector.bn_stats(fbst[:, t, :], p_sb[:, t, :])
                        nc.vector.bn_aggr(fmv[:, t, :], fbst[:, t, :])
                    fsd = gwk.tile([P, 4], F32, tag="fsd")
                    nc.scalar.activation(fsd[:], fmv[:, :, 1], AF.Sqrt,
                                         bias=C["c_eps"][:])
                    fiv = gwk.tile([P, 4], F32, tag="fiv")
                    nc.vector.reciprocal(fiv[:], fsd[:])
                    fnm = gwk.tile([P, 4], F32, tag="fnm")
                    nc.vector.scalar_tensor_tensor(
                        out=fnm[:], in0=fmv[:, :, 0], scalar=-1.0, in1=fiv[:],
                        op0=OP.mult, op1=OP.mult)
                    y1 = gwk.tile([P, 4, P], F32, tag="y1")
                    nc.vector.tensor_tensor(
                        out=y1[:], in0=p_sb[:],
                        in1=fiv[:].rearrange("p (t o) -> p t o", o=1)
                            .to_broadcast([P, 4, P]), op=OP.mult)
                    y2 = gwk.tile([P, 4, P], F32, tag="y2")
                    nc.vector.tensor_tensor(
                        out=y2[:], in0=y1[:],
                        in1=fnm[:].rearrange("p (t o) -> p t o", o=1)
                            .to_broadcast([P, 4, P]), op=OP.add)
                    yy = y2
                    if not flags["go1"]:
                        y3 = gwk.tile([P, 4, P], F32, tag="y3")
                        nc.vector.tensor_tensor(
                            out=y3[:], in0=yy[:],
                            in1=C["c_go4"][:].rearrange(
                                "p (t f) -> p t f", t=4), op=OP.mult)
                        yy = y3
                    if not flags["bo0"]:
                        y4 = gwk.tile([P, 4, P], F32, tag="y4")
                        nc.vector.tensor_tensor(
                            out=y4[:], in0=yy[:],
                            in1=C["c_bo4"][:].rearrange(
                                "p (t f) -> p t f", t=4), op=OP.add)
                        yy = y4
                    nc.scalar.dma_start(out=d_out.ap()[sl], in_=yy[:])
                if PH == 2:
                    for sl in range(SH_SLAB):
                        ws = slice(sl * 4, sl * 4 + 4)
                        dbg = gwk.tile([P, 4, P], F32, tag="dbg2")
                        nc.vector.tensor_copy(out=dbg[:], in_=v2c_nm[:, ws, :])
                        nc.scalar.dma_start(out=d_out.ap()[sl], in_=dbg[:])

    nc.compile()
    return nc


# ----------------------------------------------------------------------------
# Entry point
# ----------------------------------------------------------------------------

_CACHE = {}


def _get_kernel(cfg, flags):
    key = (cfg.N, cfg.E, cfg.S_SUB, tuple(sorted(flags.items())))
    if key not in _CACHE:
        _CACHE[key] = build_kernel(cfg, flags)
    return _CACHE[key]


def bench_hw(nc, in_maps, iters=32):
    """Build the sharded PJRT callable once; time repeated executions.

    Output buffers are zero-filled ON DEVICE each iteration (no host
    upload in the timed loop).
    """
    import time
    import jax
    from jax.sharding import Mesh, PartitionSpec, NamedSharding
    from jax.experimental.shard_map import shard_map
    import concourse.mybir as mb
    from concourse import bass2jax as b2j

    b2j.install_neuronx_cc_hook()
    n_cores = len(in_maps)
    partition_name = (nc.partition_id_tensor.name
                      if nc.partition_id_tensor else None)
    in_names, out_names, out_avals, zero_outs = [], [], [], []
    for alloc in nc.m.functions[0].allocations:
        if not isinstance(alloc, mb.MemoryLocationSet):
            continue
        name = alloc.memorylocations[0].name
        if alloc.kind == "ExternalInput":
            if name != partition_name:
                in_names.append(name)
        elif alloc.kind == "ExternalOutput":
            out_names.append(name)
            shape = tuple(alloc.tensor_shape)
            dtype = mb.dt.np(alloc.dtype)
            out_avals.append(jax.core.ShapedArray(shape, dtype))
            zero_outs.append(np.zeros(shape, dtype))
    n_params = len(in_names)
    n_outs = len(out_avals)
    in_names.extend(out_names)
    if partition_name is not None:
        in_names.append(partition_name)
    donate = tuple(range(n_params, n_params + n_outs))

    chain = max(1, int(os.environ.get("GNN_CHAIN", "1")))

    def _body(*args):
        ins = list(args[:n_params])
        outs = list(args[n_params:])
        # chain several executions per dispatch; the (fully overwritten)
        # output operands thread through so XLA cannot CSE the calls
        for _ in range(chain):
            operands = ins + outs
            if partition_name is not None:
                operands.append(b2j.partition_id_tensor())
            outs = list(b2j._bass_exec_p.bind(
                *operands,
                out_avals=tuple(out_avals), in_names=tuple(in_names),
                out_names=tuple(out_names), lowering_input_output_aliases=(),
                sim_require_finite=True, sim_require_nnan=True, nc=nc))
        return tuple(outs)

    devices = jax.devices()[:n_cores]
    mesh = Mesh(np.asarray(devices), ("core",))
    sharded = jax.jit(
        shard_map(_body, mesh=mesh,
                  in_specs=(PartitionSpec("core"),) * (n_params + n_outs),
                  out_specs=(PartitionSpec("core"),) * n_outs,
                  check_rep=False),
        donate_argnums=donate, keep_unused=True)

    concat_in = [
        np.concatenate([np.asarray(in_maps[c][in_names[i]])
                        for c in range(n_cores)], axis=0)
        for i in range(n_params)]
    in_shardings = [NamedSharding(mesh, PartitionSpec("core"))] * n_params
    in_bufs = [jax.device_put(a, s) for a, s in zip(concat_in, in_shardings)]

    import jax.numpy as jnp
    zero_sharding = tuple(
        NamedSharding(mesh, PartitionSpec("core")) for _ in range(n_outs))
    zeros_jit = jax.jit(
        lambda: tuple(
            jnp.zeros((n_cores * z.shape[0], *z.shape[1:]), z.dtype)
            for z in zero_outs),
        out_shardings=zero_sharding)

    def fresh_zeros():
        # one jitted fill per set: separate dispatches yield distinct
        # device buffers (a single batched fill gets CSE'd into one
        # buffer, which the exec's internal donation then invalidates)
        return list(zeros_jit())

    out_arrs = sharded(*in_bufs, *fresh_zeros())
    jax.block_until_ready(out_arrs)
    results = [
        {name: np.asarray(out_arrs[i]).reshape(n_cores, *out_avals[i].shape)[c]
         for i, name in enumerate(out_names)}
        for c in range(n_cores)]

    # pre-create all zero sets so the timed loop only dispatches the NEFF
    zsets = [fresh_zeros() for _ in range(iters)]
    jax.block_until_ready(zsets)

    t0 = time.perf_counter()
    outs = [sharded(*in_bufs, *z) for z in zsets]
    jax.block_until_ready(outs)
    dt = (time.perf_counter() - t0) / (iters * chain)
    return results, dt * 1e9


def kernel(**inputs):
    global LAST_EXEC_NS
    N, E = 65536, 262144
    cfg = Cfg(N, E)
    cfg, in_maps, p2o, flags = host_prep(cfg, inputs)
    nc = _get_kernel(cfg, flags)
    if bool(int(os.environ.get("GNN_BENCH", "1"))):
        results, ns = bench_hw(nc, in_maps,
                               iters=int(os.environ.get("GNN_ITERS", "32")))
        LAST_EXEC_NS = ns
    else:
        res = run_bass_kernel_spmd(nc, in_maps, core_ids=list(range(NCORES)))
        results = res.results
        LAST_EXEC_NS = res.exec_time_ns
    NSHARD = cfg.NSHARD
    y_perm = np.concatenate(
        [results[k]["y"].reshape(NSHARD // 512, P, 4, P)
         .transpose(0, 2, 1, 3).reshape(NSHARD, P)
         for k in range(NCORES)], axis=0)
    y = np.empty_like(y_perm)
    y[p2o] = y_perm
    return y.astype(np.float32)


LAST_EXEC_NS = None
